# revision 1
# baseline (speedup 1.0000x reference)
"""GCMC graph-conv kernel for Trainium2, distributed over 8 NeuronCores.

Computes: agg = segment_sum((src_feats @ W.T + b) * cj [edge_src], edge_dst) * ci

Strategy (dst-sharded, one NEFF SPMD on 8 cores):
  - Each core owns 12500 destination nodes and the edges pointing to them.
  - Phase A: each core computes wh = (X_shard @ W.T + b) * cj_shard on the
    TensorEngine and writes it (bf16) into a packed table shard: each 256B row
    holds FOUR nodes' 32-feature messages (node prow -> row prow//4, subcol
    prow%4). Packing keeps dma_gather rows at the required 256B multiple while
    the whole 25088-row table stays addressable by int16 gather indices.
  - AllGather the 8 compact shards (0.8MB each) -> full table in every HBM.
  - Phase B: edges are bucketed by (dst block, q=prow%4, dst half). Each
    128-slot tile gathers its edges' table rows (dma_gather), builds a
    one-hot matrix over its 64-dst window (is_equal on VectorE), and
    scatter-sums via PSUM-accumulating matmuls (rhs = gathered columns
    [32q : 32q+32]). Scale by ci, DMA out.

All control structure (tile counts, windows) is common across the 8 cores
(max over cores); cores pad their slots (dst_shift=-1 kills the one-hot
column; gather idx 0 is harmless).
"""
import sys

if "/opt/trn_rl_repo" not in sys.path:
    sys.path.insert(0, "/opt/trn_rl_repo")

import numpy as np
import ml_dtypes

import concourse.bacc as bacc
import concourse.mybir as mybir
import concourse.tile as tile
from concourse.bass_utils import run_bass_kernel_spmd

# problem constants (hardcoded per harness contract)
N_NODES = 100000
N_EDGES = 1_600_000
IN_DIM = 128
OUT_DIM = 32
N_CORES = 8
SHARD = N_NODES // N_CORES          # 12500 dst nodes per core
NBLK = (SHARD + 127) // 128         # 98 dst blocks per core
SPAD = NBLK * 128                   # 12544 padded shard nodes
TROWS = SPAD * N_CORES // 4         # 25088 packed table rows (4 nodes each)
LROWS = SPAD // 4                   # 3136 packed rows per core shard
ROWELEM = 128                       # bf16 elems per table row = 256B
WIN = 128                           # one-hot window: full block (PSUM base 0)
GRP = 8                             # tiles per is_equal op
BB = 5                              # dst blocks per double-buffered batch
GCAP = 25                           # tiles per dma_gather call

F32 = mybir.dt.float32
BF16 = mybir.dt.bfloat16
I16 = mybir.dt.int16


def _plan(edge_src, edge_dst):
    """Pack edges into the common SPMD structure.

    meta:
      ntiles       total tiles
      lo_of[t]     PSUM window base (0 or 64)
      q_of[t]      table subcolumn (edge prow % 4)
      toff[b]      first tile of dst block b
      batches      list of (b0, b1)
    per core:
      idx  [128, ntiles*8] int16  wrapped packed-row gather indices
      dst  [128, ntiles]   bf16   per-slot dst_shift in window (-1 = pad)
    """
    src = np.asarray(edge_src).astype(np.int64)
    dst = np.asarray(edge_dst).astype(np.int64)

    core = dst // SHARD
    dst_loc = dst % SHARD
    blk = dst_loc // 128
    dib = dst_loc % 128
    prow = (src // SHARD) * SPAD + (src % SHARD)
    row = prow // 4
    q = prow % 4

    key = ((core * NBLK + blk) * 4 + q)
    order = np.argsort(key, kind="stable")
    s_key, s_dib, s_row = key[order], dib[order], row[order]

    n_cells = N_CORES * NBLK * 4
    bounds = np.searchsorted(s_key, np.arange(n_cells + 1))

    ntiles = 0
    lo_of, q_of = [], []
    toff = np.zeros(NBLK + 1, np.int64)
    idx_cols = [[] for _ in range(N_CORES)]
    sh_cols = [[] for _ in range(N_CORES)]

    for b in range(NBLK):
        for kq in range(4):
            for h in range(1):
                segs = []
                for c in range(N_CORES):
                    cid = (c * NBLK + b) * 4 + kq
                    segs.append((int(bounds[cid]), int(bounds[cid + 1])))
                nt = (max(e - s for s, e in segs) + 127) // 128
                lo = 0
                for t in range(nt):
                    ntiles += 1
                    lo_of.append(lo)
                    q_of.append(kq)
                    for c in range(N_CORES):
                        s, e = segs[c]
                        p = s + t * 128
                        take = max(0, min(e - p, 128))
                        col_i = np.zeros(128, np.int16)
                        col_s = np.full(128, -1.0, np.float32)
                        if take > 0:
                            col_i[:take] = s_row[p:p + take]
                            col_s[:take] = s_dib[p:p + take] - lo
                        idx_cols[c].append(col_i)
                        sh_cols[c].append(col_s)
        toff[b + 1] = ntiles

    batches = [(b0, min(b0 + BB, NBLK)) for b0 in range(0, NBLK, BB)]
    meta = {"ntiles": ntiles, "lo_of": lo_of, "q_of": q_of, "toff": toff,
            "batches": batches}

    per_core = []
    for c in range(N_CORES):
        icols = np.stack(idx_cols[c], 0)          # [nt, 128]
        scols = np.stack(sh_cols[c], 0)           # [nt, 128]
        w = icols.reshape(ntiles, 8, 16).transpose(2, 0, 1).reshape(16, ntiles * 8)
        per_core.append({
            "idx": np.tile(w.astype(np.int16), (8, 1)),
            "dst": scols.T.astype(ml_dtypes.bfloat16),
        })
    return meta, per_core


def _phasea_perm():
    """Phase-A node processing order: tile t covers packed rows [32t, 32t+32);
    partition p holds local node 4*(32t + p%32) + p//32."""
    t = np.arange(SPAD) // 128
    p = np.arange(SPAD) % 128
    return 4 * (32 * t + p % 32) + p // 32


def _build(meta, mode="full", n_devices=N_CORES, no_cc=False, reps=1):
    ntiles = meta["ntiles"]
    lo_of = meta["lo_of"]
    q_of = meta["q_of"]
    toff = meta["toff"]
    batches = meta["batches"]

    nc = bacc.Bacc("TRN2", target_bir_lowering=False, debug=False,
                   enable_asserts=True, num_devices=n_devices)

    xT = nc.dram_tensor("xT", [128, SPAD], F32, kind="ExternalInput")
    wT = nc.dram_tensor("wT", [128, OUT_DIM], F32, kind="ExternalInput")
    brep = nc.dram_tensor("brep", [128, OUT_DIM], F32, kind="ExternalInput")
    cjT = nc.dram_tensor("cjT", [128, NBLK], F32, kind="ExternalInput")
    ciT = nc.dram_tensor("ciT", [128, NBLK], F32, kind="ExternalInput")
    idx_d = nc.dram_tensor("idx", [128, ntiles * 8], I16, kind="ExternalInput")
    dst_d = nc.dram_tensor("dst", [128, ntiles], BF16, kind="ExternalInput")
    out = nc.dram_tensor("out", [SPAD, OUT_DIM], F32, kind="ExternalOutput")

    gmax = 1
    for (b0, b1) in batches:
        gmax = max(gmax, int(toff[b1] - toff[b0]))

    with tile.TileContext(nc) as tc:
        with (
            tc.tile_pool(name="dram", bufs=1, space="DRAM") as dram,
            tc.tile_pool(name="const", bufs=1) as cpool,
            tc.tile_pool(name="xa", bufs=3) as xpool,
            tc.tile_pool(name="ha", bufs=3) as hpool,
            tc.tile_pool(name="wa", bufs=3) as wpool,
            tc.tile_pool(name="pa", bufs=4, space="PSUM") as ppa,
            tc.tile_pool(name="gath", bufs=2) as gpool,
            tc.tile_pool(name="smat", bufs=2) as spool,
            tc.tile_pool(name="pb", bufs=4, space="PSUM") as ppb,
            tc.tile_pool(name="res", bufs=4) as rpool,
        ):
            table_loc = dram.tile([LROWS, ROWELEM], BF16)
            table_full = dram.tile([TROWS, ROWELEM], BF16)

            # constants
            wt_t = cpool.tile([128, OUT_DIM], F32)
            nc.sync.dma_start(out=wt_t[:], in_=wT[:])
            br_t = cpool.tile([128, OUT_DIM], F32)
            nc.sync.dma_start(out=br_t[:], in_=brep[:])
            cj_t = cpool.tile([128, NBLK], F32)
            nc.sync.dma_start(out=cj_t[:], in_=cjT[:])
            ci_t = cpool.tile([128, NBLK], F32)
            nc.sync.dma_start(out=ci_t[:], in_=ciT[:])
            idx_t = cpool.tile([128, ntiles * 8], I16)
            nc.sync.dma_start(out=idx_t[:], in_=idx_d[:])
            dst_t = cpool.tile([128, ntiles], BF16)
            nc.sync.dma_start(out=dst_t[:], in_=dst_d[:])
            # iota: [128, GRP*WIN] bf16, value = col % WIN
            io_i = cpool.tile([128, GRP * WIN], I16)
            nc.gpsimd.iota(io_i[:], pattern=[[0, GRP], [1, WIN]], base=0,
                           channel_multiplier=0)
            io_b = cpool.tile([128, GRP * WIN], BF16)
            nc.vector.tensor_copy(out=io_b[:], in_=io_i[:])
            z128 = cpool.tile([128, 128], BF16)
            nc.vector.memset(z128[:], 0)
            z32 = cpool.tile([128, OUT_DIM], BF16)
            nc.vector.memset(z32[:], 0)

            # packed-table write AP: (q, r, t, f) view of [LROWS, 128]
            tab_v = table_loc[:].rearrange("(t r) (q f) -> q r t f", r=32, q=4)

            for _rep in range(reps):
                # ---- Phase A: wh = (X @ W.T + b) * cj -> packed bf16 shard ----
                ntile_a = SPAD // 128  # 98
                for a0 in range(0, ntile_a, 4):
                    an = min(4, ntile_a - a0)
                    xt = xpool.tile([128, 4 * 128], F32)
                    nc.sync.dma_start(out=xt[:, 0:an * 128],
                                      in_=xT[:, a0 * 128:(a0 + an) * 128])
                    wh4 = wpool.tile([128, 4, OUT_DIM], BF16)
                    for j in range(an):
                        ph = ppa.tile([128, OUT_DIM], F32, space="PSUM")
                        nc.tensor.matmul(out=ph[:], lhsT=xt[:, j * 128:(j + 1) * 128],
                                         rhs=wt_t[:], start=True, stop=True)
                        hb = hpool.tile([128, OUT_DIM], F32)
                        nc.vector.tensor_add(out=hb[:], in0=ph[:], in1=br_t[:])
                        nc.vector.tensor_scalar_mul(wh4[:, j, :], hb[:],
                                                    cj_t[:, a0 + j:a0 + j + 1])
                    for j in range(an):
                        nc.sync.dma_start(out=tab_v[:, :, a0 + j, :],
                                          in_=wh4[:, j, :])

                # ---- AllGather compact table shards ----
                if mode != "A" and not no_cc:
                    nc.gpsimd.collective_compute(
                        "AllGather",
                        mybir.AluOpType.bypass,
                        replica_groups=[list(range(N_CORES))],
                        ins=[table_loc.opt()],
                        outs=[table_full.opt()],
                    )

                # ---- Phase B ----
                for (b0, b1) in batches:
                    t0, t1 = int(toff[b0]), int(toff[b1])
                    tcnt = t1 - t0
                    g = gpool.tile([128, gmax, ROWELEM], BF16, tag="g")
                    s = spool.tile([128, gmax * WIN], BF16, tag="s")
                    if tcnt > 0 and mode not in ("A", "AG"):
                        for c0 in range(0, tcnt, GCAP):
                            cn = min(GCAP, tcnt - c0)
                            nc.gpsimd.dma_gather(
                                out_ap=g[:, c0:c0 + cn, :],
                                in_ap=table_full[:],
                                idxs_ap=idx_t[:, (t0 + c0) * 8:(t0 + c0 + cn) * 8],
                                num_idxs=cn * 128,
                                num_idxs_reg=cn * 128,
                                elem_size=ROWELEM,
                                single_packet=False,
                            )
                        if mode != "G":
                            for g0 in range(0, tcnt, GRP):
                                cnt = min(GRP, tcnt - g0)
                                nc.vector.tensor_tensor(
                                    out=s[:, g0 * WIN:(g0 + cnt) * WIN],
                                    in0=dst_t[:, t0 + g0:t0 + g0 + cnt, None]
                                        .to_broadcast([128, cnt, WIN]),
                                    in1=io_b[:, 0:cnt * WIN],
                                    op=mybir.AluOpType.is_equal,
                                )

                    for b in range(b0, b1):
                        acc = ppb.tile([128, OUT_DIM], F32, space="PSUM")
                        nc.tensor.matmul(out=acc[:], lhsT=z128[:], rhs=z32[:],
                                         start=True, stop=(mode != "full"),
                                         skip_group_check=True)
                        if mode == "full":
                            tb0, tb1 = int(toff[b]), int(toff[b + 1])
                            for t in range(tb0, tb1):
                                gi = t - t0
                                lo = lo_of[t]
                                kq = q_of[t]
                                nc.tensor.matmul(
                                    out=acc[lo:lo + WIN, :],
                                    lhsT=s[:, gi * WIN:(gi + 1) * WIN],
                                    rhs=g[:, gi, 32 * kq:32 * kq + OUT_DIM],
                                    start=False, stop=(t == tb1 - 1),
                                    skip_group_check=True,
                                )
                        res = rpool.tile([128, OUT_DIM], F32)
                        nc.vector.tensor_scalar_mul(res[:], acc[:], ci_t[:, b:b + 1])
                        nc.sync.dma_start(out=out[b * 128:(b + 1) * 128, :], in_=res[:])
    nc.compile()
    return nc


def _in_maps(ins, per_core):
    src_feats = np.ascontiguousarray(np.asarray(ins["src_feats"], dtype=np.float32))
    cj = np.asarray(ins["cj"], dtype=np.float32).reshape(-1)
    ci = np.asarray(ins["ci"], dtype=np.float32).reshape(-1)
    W = np.asarray(ins["W"], dtype=np.float32)
    b = np.asarray(ins["b"], dtype=np.float32).reshape(-1)

    perm = _phasea_perm()
    maps = []
    for c in range(N_CORES):
        lo, hi = c * SHARD, (c + 1) * SHARD
        xf = np.zeros((SPAD, IN_DIM), np.float32)
        xf[:SHARD] = src_feats[lo:hi]
        cjf = np.zeros(SPAD, np.float32)
        cjf[:SHARD] = cj[lo:hi]
        cif = np.zeros(SPAD, np.float32)
        cif[:SHARD] = ci[lo:hi]
        xP = xf[perm]            # phase-A processing order
        cjP = cjf[perm]
        m = {
            "xT": np.ascontiguousarray(xP.T),
            "wT": np.ascontiguousarray(W.T),
            "brep": np.tile(b[None, :], (128, 1)),
            "cjT": np.ascontiguousarray(cjP.reshape(NBLK, 128).T),
            "ciT": np.ascontiguousarray(cif.reshape(NBLK, 128).T),
        }
        m.update(per_core[c])
        maps.append(m)
    return maps


def kernel(src_feats, cj, ci, W, b, edge_src, edge_dst):
    ins = {"src_feats": src_feats, "cj": cj, "ci": ci, "W": W, "b": b}
    meta, per_core = _plan(edge_src, edge_dst)
    nc = _build(meta)
    maps = _in_maps(ins, per_core)
    res = run_bass_kernel_spmd(nc, maps, core_ids=list(range(N_CORES)))
    outs = [res.results[c]["out"][:SHARD] for c in range(N_CORES)]
    return np.concatenate(outs, 0).astype(np.float32)



# revision 2
# speedup vs baseline: 1.2766x; 1.2766x over previous
"""GCMC graph-conv kernel for Trainium2, distributed over 8 NeuronCores.

Computes: agg = segment_sum((src_feats @ W.T + b) * cj [edge_src], edge_dst) * ci

Strategy (dst-sharded, gather-raw-features, W-after-aggregate):
  By linearity, sum_e cj_s*(W x_s + b) = W * (sum_e cj_s x_s) + b * (sum_e cj_s).
  So we gather RAW (cj-prescaled, bf16) 128-dim feature rows per edge --
  exactly one 256B DMA-gather element, all bytes useful -- scatter-sum them
  into per-dst-block [128 feat x 128 dst] PSUM accumulators via one-hot
  matmuls, and apply W once per aggregated dst block. No per-core transform
  phase and no AllGather (every core reads the same replicated table).

  - Host: table x' = (src_feats * cj) as bf16 [100352, 128]; rows split in
    4 chunks of 25088 so gather indices fit int16. Edges of core c = those
    with dst in [12500c, 12500(c+1)); bucketed by (batch of 5 dst blocks,
    src chunk Q, dst block); each 128-slot tile gathers its edges' rows
    (dma_gather on queue Q -- 4 SWDGE queues in parallel), builds a one-hot
    [edge x dst-in-block] with is_equal, and accumulates
    acc[feat, dst] += gathered^T @ onehot via PSUM-accumulating matmuls.
  - Per block: copy acc PSUM->SBUF, res = (acc^T W^T) * ci, DMA out.

All control structure (tile counts, ranges) is common across the 8 cores
(max over cores); cores pad their slots (dst = -1 kills the one-hot column;
gather idx 0 is harmless).
"""
import sys

if "/opt/trn_rl_repo" not in sys.path:
    sys.path.insert(0, "/opt/trn_rl_repo")

import numpy as np
import ml_dtypes

import concourse.bacc as bacc
import concourse.mybir as mybir
import concourse.tile as tile
from concourse.bass_utils import run_bass_kernel_spmd

# problem constants (hardcoded per harness contract)
N_NODES = 100000
N_EDGES = 1_600_000
IN_DIM = 128
OUT_DIM = 32
N_CORES = 8
SHARD = N_NODES // N_CORES          # 12500 dst nodes per core
NBLK = (SHARD + 127) // 128         # 98 dst blocks per core
SPAD = NBLK * 128                   # 12544 padded shard nodes
NCHUNK = 4                          # table chunks (int16 gather index range)
CHUNK = 25088                       # nodes per chunk; 4*25088 = 100352
TROWS = NCHUNK * CHUNK
WIN = 128                           # one-hot window = dst block
GRP = 16                            # tiles per is_equal op
BB = 5                              # dst blocks per batch
NBATCH = (NBLK + BB - 1) // BB      # 20
GCAP = 49                           # max tiles per dma_gather call

F32 = mybir.dt.float32
BF16 = mybir.dt.bfloat16
I16 = mybir.dt.int16
I8 = mybir.dt.int8
F8 = mybir.dt.float8e4


def _plan(edge_src, edge_dst, bb=BB, half=True, pack=True, pairb=False,
          sort_rows=True):
    """Bucket edges into the common SPMD tile structure.

    meta:
      ntiles           total tiles
      tiles_of_block   per dst block: list of (tile id, base, size) --
                       the matmul for this block contracts over tile
                       partitions [base, base+size)
      batches          per batch: dict(blocks=(b0,b1), span=(t0,t1),
                                       qruns=[(Q, q0, q1), ...])
      gmax             max tiles per batch
    per core:
      idx  [128, ntiles*8] int16  wrapped chunk-row gather indices
      dst_raw [128, ntiles] f32   per-slot dst-in-block (-1 = pad)
      srcs [ntiles, 128]   int32  per-slot global src node (-1 = pad)

    half=True packs two 64-slot half-tiles (possibly of different dst
    blocks, same chunk) per 128-slot gather tile, cutting ceil padding.
    """
    src = np.asarray(edge_src).astype(np.int64)
    dst = np.asarray(edge_dst).astype(np.int64)

    core = dst // SHARD
    dloc = dst % SHARD
    blk = dloc // 128
    dib = dloc % 128
    q = src // CHUNK
    row = src % CHUNK
    batch = blk // bb
    nbatch = (NBLK + bb - 1) // bb

    key = ((core * nbatch + batch) * NCHUNK + q) * NBLK + blk
    if sort_rows:
        # secondary sort by table row: ascending rows within each gather
        # call improve DRAM row-buffer locality
        order = np.lexsort((row, key))
    else:
        order = np.argsort(key, kind="stable")
    s_key, s_dib, s_row, s_src = key[order], dib[order], row[order], src[order]

    n_cells = N_CORES * nbatch * NCHUNK * NBLK
    bounds = np.searchsorted(s_key, np.arange(n_cells + 1))

    def cellid(c, bt, kq, b):
        return ((c * nbatch + bt) * NCHUNK + kq) * NBLK + b

    ntiles = 0
    tiles_of_block = [[] for _ in range(NBLK)]
    batches = []
    idx_cols = [[] for _ in range(N_CORES)]
    sh_cols = [[] for _ in range(N_CORES)]
    src_cols = [[] for _ in range(N_CORES)]

    HS = 64 if half else 128  # slots per (block-aligned) sub-tile
    PER = (128 // HS) if pack else 1  # sub-tiles packed per gather tile

    for bt in range(nbatch):
        b0, b1 = bt * bb, min(bt * bb + bb, NBLK)
        t_start = ntiles
        qruns = []
        for kq in range(NCHUNK):
            q0 = ntiles
            # sub-tiles of this (batch, chunk) run: (block, part_index)
            subs = []
            segs_of = {}
            for b in range(b0, b1):
                segs = []
                for c in range(N_CORES):
                    cid = cellid(c, bt, kq, b)
                    segs.append((int(bounds[cid]), int(bounds[cid + 1])))
                segs_of[b] = segs
                nh = (max(e - s for s, e in segs) + HS - 1) // HS
                subs.extend((b, j) for j in range(nh))
            if pairb and half:
                # pair adjacent blocks: block A's halves all in band 0,
                # block B's all in band 64 -- one PSUM acc per block
                nh_of = {}
                for b in range(b0, b1):
                    nh_of[b] = max((j for (bx, j) in subs if bx == b),
                                   default=-1) + 1
                packs = []
                blist = list(range(b0, b1))
                for pi in range(0, len(blist), 2):
                    pr = blist[pi:pi + 2]
                    nt = max(nh_of[b] for b in pr)
                    for j in range(nt):
                        packs.append([(pr[k], j) for k in range(len(pr))
                                      if j < nh_of[pr[k]]])
            else:
                packs = [subs[i0:i0 + PER]
                         for i0 in range(0, len(subs), PER)]
            for pk in packs:
                cols_i = [np.zeros(128, np.int16) for _ in range(N_CORES)]
                cols_s = [np.full(128, -1.0, np.float32)
                          for _ in range(N_CORES)]
                cols_n = [np.full(128, -1, np.int32) for _ in range(N_CORES)]
                for (b, j) in pk:
                    base = (b - b0) % 2 * HS if (pairb and half) else \
                        pk.index((b, j)) * HS
                    tiles_of_block[b].append((ntiles, base, HS))
                    for c in range(N_CORES):
                        s, e = segs_of[b][c]
                        p = s + j * HS
                        take = max(0, min(e - p, HS))
                        if take > 0:
                            cols_i[c][base:base + take] = s_row[p:p + take]
                            cols_s[c][base:base + take] = s_dib[p:p + take]
                            cols_n[c][base:base + take] = s_src[p:p + take]
                for c in range(N_CORES):
                    idx_cols[c].append(cols_i[c])
                    sh_cols[c].append(cols_s[c])
                    src_cols[c].append(cols_n[c])
                ntiles += 1
            if ntiles > q0:
                qruns.append((kq, q0, ntiles))
        batches.append({"blocks": (b0, b1), "span": (t_start, ntiles),
                        "qruns": qruns})

    gmax = max(b["span"][1] - b["span"][0] for b in batches)
    meta = {"ntiles": ntiles, "tiles_of_block": tiles_of_block,
            "batches": batches, "gmax": gmax}

    per_core = []
    for c in range(N_CORES):
        icols = np.stack(idx_cols[c], 0)          # [nt, 128]
        scols = np.stack(sh_cols[c], 0)           # [nt, 128]
        w = icols.reshape(ntiles, 8, 16).transpose(2, 0, 1).reshape(16, ntiles * 8)
        per_core.append({
            "idx": np.tile(w.astype(np.int16), (8, 1)),
            "dst_raw": np.ascontiguousarray(scols.T),
            "srcs": np.stack(src_cols[c], 0),
        })
    return meta, per_core


def _build(meta, n_devices=N_CORES, reps=1, has_bias=False, hw_loop=False,
           mode="full", oh8=False, gcap=GCAP, queue_by_chunk=True,
           gbufs=3, sbufs=3, grp=GRP, finish_act=False, add_gps=False,
           single_packet=False, asserts=False):
    do_gather = mode in ("full", "G", "GE")
    do_onehot = mode in ("full", "E", "GE")
    do_matmul = mode in ("full", "M")
    cmp_dt = I8 if oh8 else BF16
    oh_dt = F8 if oh8 else BF16
    ntiles = meta["ntiles"]
    tiles_of_block = meta["tiles_of_block"]
    batches = meta["batches"]
    gmax = meta["gmax"]

    nc = bacc.Bacc("TRN2", target_bir_lowering=False, debug=False,
                   enable_asserts=asserts, num_devices=n_devices,
                   num_swdge_queues=4)

    tab = nc.dram_tensor("tab", [TROWS, IN_DIM], BF16, kind="ExternalInput")
    wT = nc.dram_tensor("wT", [128, OUT_DIM], F32, kind="ExternalInput")
    ciT = nc.dram_tensor("ciT", [128, NBLK], F32, kind="ExternalInput")
    idx_d = nc.dram_tensor("idx", [128, ntiles * 8], I16, kind="ExternalInput")
    dst_d = nc.dram_tensor("dst", [128, ntiles], cmp_dt, kind="ExternalInput")
    if has_bias:
        br_d = nc.dram_tensor("brep", [128, OUT_DIM], F32, kind="ExternalInput")
        cjs_d = nc.dram_tensor("cjs", [128, ntiles], BF16, kind="ExternalInput")
    out = nc.dram_tensor("out", [SPAD, OUT_DIM], F32, kind="ExternalOutput")

    with tile.TileContext(nc) as tc:
        with (
            tc.tile_pool(name="const", bufs=1) as cpool,
            tc.tile_pool(name="gath", bufs=gbufs) as gpool,
            tc.tile_pool(name="smat", bufs=sbufs) as spool,
            tc.tile_pool(name="pacc", bufs=6, space="PSUM") as pacc,
            tc.tile_pool(name="pres", bufs=2, space="PSUM") as pres,
            tc.tile_pool(name="atsb", bufs=3) as apool,
            tc.tile_pool(name="res", bufs=3) as rpool,
        ):
            wt_t = cpool.tile([128, OUT_DIM], F32)
            nc.sync.dma_start(out=wt_t[:], in_=wT[:])
            ci_t = cpool.tile([128, NBLK], F32)
            nc.sync.dma_start(out=ci_t[:], in_=ciT[:])
            idx_t = cpool.tile([128, ntiles * 8], I16)
            nc.sync.dma_start(out=idx_t[:], in_=idx_d[:])
            dst_t = cpool.tile([128, ntiles], cmp_dt)
            nc.sync.dma_start(out=dst_t[:], in_=dst_d[:])
            if has_bias:
                br_t = cpool.tile([128, OUT_DIM], F32)
                nc.sync.dma_start(out=br_t[:], in_=br_d[:])
                cjs_t = cpool.tile([128, ntiles], BF16)
                nc.sync.dma_start(out=cjs_t[:], in_=cjs_d[:])
            io_i = cpool.tile([128, grp * WIN], I16)
            nc.gpsimd.iota(io_i[:], pattern=[[0, grp], [1, WIN]], base=0,
                           channel_multiplier=0)
            io_b = cpool.tile([128, grp * WIN], cmp_dt)
            nc.vector.tensor_copy(out=io_b[:], in_=io_i[:])
            g_const = s_const = None
            if do_matmul and not do_gather:
                g_const = cpool.tile([128, gmax, IN_DIM], BF16)
                nc.vector.memset(g_const[:], 0)
            if do_matmul and not do_onehot:
                s_const = cpool.tile([128, gmax * WIN], oh_dt)
                nc.vector.memset(s_const[:], 0)

            def body():
                pending = None  # (block, acc psum tile, cjacc or None)

                def finish(pend):
                    b, acc, acc1, cja = pend
                    at = apool.tile([128, 128], F32)
                    if finish_act:
                        nc.scalar.copy(out=at[:], in_=acc[:])
                    else:
                        nc.vector.tensor_copy(out=at[:], in_=acc[:])
                    if acc1 is not None:
                        eng = nc.gpsimd if add_gps else nc.vector
                        eng.tensor_tensor(out=at[:], in0=at[:], in1=acc1[:],
                                          op=mybir.AluOpType.add)
                    rp = pres.tile([128, OUT_DIM], F32, space="PSUM")
                    nc.tensor.matmul(out=rp[:], lhsT=at[:], rhs=wt_t[:],
                                     start=True, stop=True)
                    res = rpool.tile([128, OUT_DIM], F32)
                    if has_bias:
                        tmp = rpool.tile([128, OUT_DIM], F32)
                        nc.vector.tensor_scalar_mul(tmp[:], br_t[:],
                                                    cja[:, 0:1])
                        nc.vector.tensor_add(out=tmp[:], in0=rp[:], in1=tmp[:])
                        nc.vector.tensor_scalar_mul(res[:], tmp[:],
                                                    ci_t[:, b:b + 1])
                    elif finish_act:
                        nc.scalar.mul(out=res[:], in_=rp[:],
                                      mul=ci_t[:, b:b + 1])
                    else:
                        nc.vector.tensor_scalar_mul(res[:], rp[:],
                                                    ci_t[:, b:b + 1])
                    nc.sync.dma_start(out=out[b * 128:(b + 1) * 128, :],
                                      in_=res[:])

                for binfo in batches:
                    t0, t1 = binfo["span"]
                    tcnt = t1 - t0
                    if tcnt == 0:
                        continue
                    if do_gather:
                        g = gpool.tile([128, gmax, IN_DIM], BF16, tag="g",
                                       name="g")
                    else:
                        g = g_const
                    if do_onehot:
                        s = spool.tile([128, gmax * WIN], oh_dt, tag="s",
                                       name="s")
                    else:
                        s = s_const
                    for (kq, q0, q1) in (binfo["qruns"] if do_gather else []):
                        for c0 in range(q0, q1, gcap):
                            cn = min(gcap, q1 - c0)
                            nc.gpsimd.dma_gather(
                                out_ap=g[:, c0 - t0:c0 - t0 + cn, :],
                                in_ap=tab[kq * CHUNK:(kq + 1) * CHUNK, :],
                                idxs_ap=idx_t[:, c0 * 8:(c0 + cn) * 8],
                                num_idxs=cn * 128,
                                num_idxs_reg=cn * 128,
                                elem_size=IN_DIM,
                                single_packet=single_packet,
                                queue_num=kq if queue_by_chunk else 0,
                            )
                    for g0 in (range(t0, t1, grp) if do_onehot else []):
                        cnt = min(grp, t1 - g0)
                        nc.vector.tensor_tensor(
                            out=s[:, (g0 - t0) * WIN:(g0 - t0 + cnt) * WIN],
                            in0=dst_t[:, g0:g0 + cnt, None]
                                .to_broadcast([128, cnt, WIN]),
                            in1=io_b[:, 0:cnt * WIN],
                            op=mybir.AluOpType.is_equal,
                        )
                    for b in (range(*binfo["blocks"]) if do_matmul else []):
                        tl = tiles_of_block[b]
                        # one PSUM accumulator per PE row band: concurrent
                        # row-group matmuls must not accumulate into the
                        # same PSUM region (HW race -> engine hang)
                        bands = sorted({base for (_, base, _) in tl})
                        accs = {}
                        for bi, band in enumerate(bands):
                            btl = [e for e in tl if e[1] == band]
                            acc = pacc.tile([128, 128], F32, space="PSUM",
                                            name="acc")
                            accs[band] = acc
                            for i, (t, base, size) in enumerate(btl):
                                gi = t - t0
                                nc.tensor.matmul(
                                    out=acc[:],
                                    lhsT=g[base:base + size, gi, :],
                                    rhs=s[base:base + size,
                                          gi * WIN:(gi + 1) * WIN],
                                    start=(i == 0), stop=(i == len(btl) - 1),
                                    skip_group_check=True,
                                )
                        cja = None
                        if has_bias:
                            cja = pacc.tile([128, 1], F32, space="PSUM",
                                            name="cja")
                            for i, (t, base, size) in enumerate(tl):
                                gi = t - t0
                                nc.tensor.matmul(
                                    out=cja[:],
                                    lhsT=s[base:base + size,
                                           gi * WIN:(gi + 1) * WIN],
                                    rhs=cjs_t[base:base + size, t:t + 1],
                                    start=(i == 0), stop=(i == len(tl) - 1),
                                    skip_group_check=True,
                                )
                        if pending is not None:
                            finish(pending)
                        acc_l = [accs[band] for band in bands]
                        pending = (b, acc_l[0],
                                   acc_l[1] if len(acc_l) > 1 else None, cja)
                if pending is not None:
                    finish(pending)
                    pending = None

            if hw_loop and reps > 1:
                with tc.For_i(0, reps, 1):
                    body()
            else:
                for _rep in range(reps):
                    body()
    nc.compile()
    return nc


def _in_maps(ins, per_core, has_bias=False, oh8=False):
    src_feats = np.asarray(ins["src_feats"], dtype=np.float32)
    cj = np.asarray(ins["cj"], dtype=np.float32).reshape(-1)
    ci = np.asarray(ins["ci"], dtype=np.float32).reshape(-1)
    W = np.asarray(ins["W"], dtype=np.float32)
    b = np.asarray(ins["b"], dtype=np.float32).reshape(-1)

    tabf = np.zeros((TROWS, IN_DIM), np.float32)
    tabf[:N_NODES] = src_feats * cj[:, None]
    tab = np.ascontiguousarray(tabf.astype(ml_dtypes.bfloat16))
    wt = np.ascontiguousarray(W.T)

    maps = []
    for c in range(N_CORES):
        lo, hi = c * SHARD, (c + 1) * SHARD
        cif = np.zeros(SPAD, np.float32)
        cif[:SHARD] = ci[lo:hi]
        draw = per_core[c]["dst_raw"]
        m = {
            "tab": tab,
            "wT": wt,
            "ciT": np.ascontiguousarray(cif.reshape(NBLK, 128).T),
            "idx": per_core[c]["idx"],
            "dst": np.ascontiguousarray(
                draw.astype(np.int8) if oh8
                else draw.astype(ml_dtypes.bfloat16)),
        }
        if has_bias:
            srcs = per_core[c]["srcs"]            # [ntiles, 128] int32
            cjsl = np.where(srcs >= 0, cj[np.clip(srcs, 0, N_NODES - 1)], 0.0)
            m["cjs"] = np.ascontiguousarray(
                cjsl.T.astype(ml_dtypes.bfloat16))
            m["brep"] = np.tile(b[None, :], (128, 1)).astype(np.float32)
        maps.append(m)
    return maps


def kernel(src_feats, cj, ci, W, b, edge_src, edge_dst):
    ins = {"src_feats": src_feats, "cj": cj, "ci": ci, "W": W, "b": b}
    has_bias = bool(np.any(np.asarray(b) != 0))
    meta, per_core = _plan(edge_src, edge_dst, half=not has_bias)
    nc = _build(meta, has_bias=has_bias)
    maps = _in_maps(ins, per_core, has_bias=has_bias)
    res = run_bass_kernel_spmd(nc, maps, core_ids=list(range(N_CORES)))
    outs = [res.results[c]["out"][:SHARD] for c in range(N_CORES)]
    return np.concatenate(outs, 0).astype(np.float32)
